# revision 1
# baseline (speedup 1.0000x reference)
"""Trainium2 Bass kernel for a 2-layer GCN + question-broadcast log_softmax.

Reference computation (see problem statement):
    h1  = relu(gcnconv(x, W1, b1));  h2 = gcnconv(h1, W2, b2)
    out[b, i, :] = log_softmax(h2[i, :] + q[b, :], axis=nodes)
q[b, :] is constant along the softmax axis, so it cancels exactly:
    out[b] = h2 - logsumexp(h2, axis=0)   (identical for every b)

Strategy (8 NeuronCores, 1D graph partition by destination node):
  * Nodes are sharded 12500/core (padded to 12544 = 98 windows x 128).
  * Per core, incident edges (by dest) are bucketed into (window, quarter)
    cells: window = 128 consecutive local dest nodes, quarter = 1 of 4
    source-row ranges of <= 25088 rows (dma_gather indices are int16).
  * Per layer: y = (x @ W) * dinv[row] is computed shard-wise in bf16 and
    AllGather'ed; messages y[row] are fetched with dma_gather (256B/edge);
    segment-sum into each window is a tensor-engine matmul with an
    indicator matrix S[msg, dest] built on the vector engine by comparing
    the dest id stream against an iota row (S = (iota == dest)).
  * PSUM accumulates a [128 dest x 128 feat] tile per window; the flush
    applies the dinv[dest] scale, self-loop term, bias and ReLU.
  * log-softmax epilogue: per-feature max/sum-exp reductions via PE
    transpose + AllReduce across cores, then a broadcasted subtract.

kernel() is self-contained: it hardcodes shapes, preprocesses the integer
graph structure on host (partitioning/sorting/padding only), compiles the
Bass program once per edge-structure, and runs it on 8 cores via
run_bass_kernel_spmd.
"""

import os
import sys
import hashlib
import numpy as np

sys.path.insert(0, "/opt/trn_rl_repo")

import ml_dtypes  # noqa: E402

BF16 = ml_dtypes.bfloat16

TILE = 128


class Cfg:
    def __init__(self, n_nodes, n_edges, d=128, ncores=8):
        assert n_nodes % ncores == 0
        self.N = n_nodes
        self.E = n_edges
        self.D = d
        self.NC = ncores
        self.NPC = n_nodes // ncores                    # real nodes per core
        self.W = (self.NPC + TILE - 1) // TILE          # windows per core
        self.NPCP = self.W * TILE                       # padded nodes per core
        total_padded = self.NPCP * ncores
        # quarters of the (padded) global row space; must fit int16 indices
        self.NQ = 1
        while total_padded // self.NQ > 32768 or total_padded % self.NQ:
            self.NQ += 1
        self.QROWS = total_padded // self.NQ
        assert self.QROWS <= 32768
        self.last_w = self.NPC - (self.W - 1) * TILE    # dests in last window


FULL = Cfg(100000, 3200000)

DEST_SENTINEL = 200.0  # compare-miss value for padded message slots


# --------------------------------------------------------------------------
# host-side integer preprocessing: partition, bucket, sort, pad
# --------------------------------------------------------------------------

def prepare(cfg, x, edge_index, W1, b1, W2, b2):
    N, D, NC = cfg.N, cfg.D, cfg.NC
    NPC, NPCP, W, NQ, QROWS = cfg.NPC, cfg.NPCP, cfg.W, cfg.NQ, cfg.QROWS

    row = np.asarray(edge_index[0], dtype=np.int64)
    col = np.asarray(edge_index[1], dtype=np.int64)

    deg = np.bincount(col, minlength=N).astype(np.float32) + 1.0

    core = col // NPC
    wl = (col % NPC) // TILE
    drel = (col % NPC) % TILE
    grow = (row // NPC) * NPCP + (row % NPC)            # padded global row
    q = grow // QROWS
    lidx = (grow - q * QROWS).astype(np.int64)

    order = np.lexsort((drel, q, wl, core))
    core_s, wl_s, q_s, drel_s, lidx_s = (
        core[order], wl[order], q[order], drel[order], lidx[order])

    # per (core, window, quarter) cell counts
    cell = (core_s * W + wl_s) * NQ + q_s
    counts = np.bincount(cell, minlength=NC * W * NQ).reshape(NC, W, NQ)
    T = np.ceil(counts.max(axis=0) / TILE).astype(np.int64)  # [W, NQ] tiles
    ntw = T.sum(axis=1)                                      # [W] tiles/window
    NTT = int(T.sum())
    L = NTT * TILE

    # stream offsets (slots) for every cell, shared by all cores
    cell_off = np.zeros(W * NQ + 1, dtype=np.int64)
    cell_off[1:] = np.cumsum(T.reshape(-1) * TILE)

    # per-core fill positions
    cs = counts.reshape(NC, -1)
    core_cell = core_s * (W * NQ) + (wl_s * NQ + q_s)
    starts = np.zeros(NC * W * NQ + 1, dtype=np.int64)
    starts[1:] = np.cumsum(cs.reshape(-1))
    rank = np.arange(len(order)) - starts[core_cell]
    pos = cell_off[wl_s * NQ + q_s] + rank               # slot within core

    idx_arr = np.zeros((NC, L), dtype=np.int16)          # pad -> row 0 (valid)
    dest_arr = np.full((NC, L), DEST_SENTINEL, dtype=np.float32)
    idx_arr[core_s, pos] = lidx_s.astype(np.int16)
    dest_arr[core_s, pos] = drel_s.astype(np.float32)

    # wrapped int16 index layout for dma_gather: G[p, s] = idx[s*16 + p%16],
    # replicated across the 8 groups of 16 partitions
    idx_w = idx_arr.reshape(NC, L // 16, 16).transpose(0, 2, 1)   # [NC,16,L/16]
    idx_w = np.tile(idx_w, (1, 8, 1)).copy()                      # [NC,128,L/16]

    # dest stream transposed: dest_t[p, t] = dest[t*128 + p]
    dest_t = dest_arr.reshape(NC, NTT, TILE).transpose(0, 2, 1).astype(BF16).copy()

    # per-core padded node data
    x = np.asarray(x, dtype=np.float32)
    x_sh = np.zeros((NC, NPCP, D), dtype=np.float32)
    deg_sh = np.ones((NC, NPCP), dtype=np.float32)
    for c in range(NC):
        x_sh[c, :NPC] = x[c * NPC:(c + 1) * NPC]
        deg_sh[c, :NPC] = deg[c * NPC:(c + 1) * NPC]
    deg_t = deg_sh.reshape(NC, W, TILE).transpose(0, 2, 1).copy()  # [NC,128,W]

    iota = np.broadcast_to(np.arange(TILE, dtype=np.float32), (TILE, TILE))
    iota = iota.astype(BF16).copy()

    W1 = np.asarray(W1, np.float32)
    W2 = np.asarray(W2, np.float32)
    b1r = np.asarray(b1, np.float32).reshape(1, D)
    b2r = np.asarray(b2, np.float32).reshape(1, D)

    in_maps = []
    for c in range(NC):
        in_maps.append({
            "x_sh": x_sh[c],
            "deg_t": deg_t[c],
            "w1": W1, "w2": W2, "b1r": b1r, "b2r": b2r,
            "iota": iota,
            "idx_w": idx_w[c],
            "dest_t": dest_t[c],
        })

    meta = {
        "T": tuple(map(tuple, T.tolist())),
        "ntw": tuple(ntw.tolist()),
        "NTT": NTT,
    }
    return in_maps, meta


# --------------------------------------------------------------------------
# Bass program
# --------------------------------------------------------------------------

def build(cfg, meta, stage=4):
    import concourse.bacc as bacc
    import concourse.mybir as mybir
    from concourse.tile import TileContext
    from concourse.masks import make_identity

    f32 = mybir.dt.float32
    bf16 = mybir.dt.bfloat16
    i16 = mybir.dt.int16
    AF = mybir.ActivationFunctionType
    OP = mybir.AluOpType

    D, W, NPCP, NQ, QROWS, NC = cfg.D, cfg.W, cfg.NPCP, cfg.NQ, cfg.QROWS, cfg.NC
    T = meta["T"]
    ntw = meta["ntw"]
    NTT = meta["NTT"]
    L16 = NTT * TILE // 16
    maxtw = max(ntw) if NTT else 1

    nc = bacc.Bacc(None, target_bir_lowering=False, debug=False)

    x_sh = nc.dram_tensor("x_sh", [NPCP, D], f32, kind="ExternalInput")
    deg_t = nc.dram_tensor("deg_t", [TILE, W], f32, kind="ExternalInput")
    w1 = nc.dram_tensor("w1", [D, D], f32, kind="ExternalInput")
    w2 = nc.dram_tensor("w2", [D, D], f32, kind="ExternalInput")
    b1r = nc.dram_tensor("b1r", [1, D], f32, kind="ExternalInput")
    b2r = nc.dram_tensor("b2r", [1, D], f32, kind="ExternalInput")
    iota_in = nc.dram_tensor("iota", [TILE, TILE], bf16, kind="ExternalInput")
    idx_w = nc.dram_tensor("idx_w", [TILE, L16], i16, kind="ExternalInput")
    dest_t = nc.dram_tensor("dest_t", [TILE, NTT], bf16, kind="ExternalInput")

    out_sh = nc.dram_tensor("out_sh", [NPCP, D], f32, kind="ExternalOutput")

    y1_sh = nc.dram_tensor("y1_sh", [NPCP, D], bf16)
    y2_sh = nc.dram_tensor("y2_sh", [NPCP, D], bf16)
    y1_full = nc.dram_tensor("y1_full", [NC * NPCP, D], bf16, addr_space="Shared")
    y2_full = nc.dram_tensor("y2_full", [NC * NPCP, D], bf16, addr_space="Shared")
    h2_sh = nc.dram_tensor("h2_sh", [NPCP, D], f32)
    red_in = nc.dram_tensor("red_in", [1, D], f32)
    red_max = nc.dram_tensor("red_max", [1, D], f32, addr_space="Shared")
    red_in2 = nc.dram_tensor("red_in2", [1, D], f32)
    red_sum = nc.dram_tensor("red_sum", [1, D], f32, addr_space="Shared")

    groups = [list(range(NC))]

    with TileContext(nc) as tc:
        with (
            tc.tile_pool(name="const", bufs=1) as constp,
            tc.tile_pool(name="slp", bufs=1) as slp,
            tc.tile_pool(name="io", bufs=4) as iop,
            tc.tile_pool(name="msg", bufs=3) as msgp,
            tc.tile_pool(name="sp", bufs=3) as sp,
            tc.tile_pool(name="flush", bufs=4) as flp,
            tc.tile_pool(name="acc", bufs=1) as accp,
            tc.tile_pool(name="mm", bufs=3, space="PSUM") as mmp,
            tc.tile_pool(name="tr", bufs=2, space="PSUM") as trp,
        ):
            # ---------------- constants ----------------
            ident = constp.tile([TILE, TILE], f32)
            make_identity(nc, ident[:])
            iota_t = constp.tile([TILE, TILE], bf16)
            nc.sync.dma_start(out=iota_t[:], in_=iota_in[:, :])
            w1_t = constp.tile([D, D], f32)
            nc.sync.dma_start(out=w1_t[:], in_=w1[:, :])
            w2_t = constp.tile([D, D], f32)
            nc.sync.dma_start(out=w2_t[:], in_=w2[:, :])
            ones_row = constp.tile([1, TILE], f32)
            nc.vector.memset(ones_row[:], 1.0)

            # bias rows broadcast to [128, D] via K=1 matmul
            b_bc = []
            for name, bt in (("b1", b1r), ("b2", b2r)):
                brow = constp.tile([1, D], f32, tag=f"{name}row")
                nc.sync.dma_start(out=brow[:], in_=bt[:, :])
                bps = trp.tile([TILE, D], f32, tag="tr")
                nc.tensor.matmul(bps[:], lhsT=ones_row[:], rhs=brow[:],
                                 start=True, stop=True)
                bsb = constp.tile([TILE, D], f32, tag=f"{name}bc")
                nc.vector.tensor_copy(out=bsb[:], in_=bps[:])
                b_bc.append(bsb)
            b1_bc, b2_bc = b_bc

            # dinv / dinv^2 per local node, window-major [128, W]
            degs = constp.tile([TILE, W], f32)
            nc.sync.dma_start(out=degs[:], in_=deg_t[:, :])
            rdeg = constp.tile([TILE, W], f32)
            nc.vector.reciprocal(rdeg[:], degs[:])
            dinv = constp.tile([TILE, W], f32)
            nc.scalar.sqrt(dinv[:], rdeg[:])
            dinv2 = constp.tile([TILE, W], f32)
            nc.vector.tensor_tensor(out=dinv2[:], in0=dinv[:], in1=dinv[:],
                                    op=OP.mult)

            # self-loop + bias term, kept resident in SBUF: [128, W*128]
            sl = slp.tile([TILE, W * D], f32)

            # ---------------- prologue: xw, y1, sl1 ----------------
            for w in range(W):
                x_t = iop.tile([TILE, D], f32, tag="x")
                nc.sync.dma_start(out=x_t[:], in_=x_sh[w * TILE:(w + 1) * TILE, :])
                xT_ps = trp.tile([TILE, D], f32, tag="tr")
                nc.tensor.transpose(xT_ps[:], x_t[:], ident[:])
                xT = iop.tile([TILE, D], f32, tag="xT")
                nc.vector.tensor_copy(out=xT[:], in_=xT_ps[:])
                xw_ps = mmp.tile([TILE, D], f32, tag="agg")
                nc.tensor.matmul(xw_ps[:], lhsT=xT[:], rhs=w1_t[:],
                                 start=True, stop=True)
                # sl1 = xw * dinv2 + b1
                slw = sl[:, w * D:(w + 1) * D]
                nc.vector.tensor_scalar(out=slw, in0=xw_ps[:],
                                        scalar1=dinv2[:, w:w + 1], scalar2=None,
                                        op0=OP.mult)
                nc.vector.tensor_tensor(out=slw, in0=slw, in1=b1_bc[:], op=OP.add)
                # y1 = xw * dinv (bf16)
                y_t = iop.tile([TILE, D], bf16, tag="y")
                nc.scalar.activation(y_t[:], xw_ps[:], AF.Copy,
                                     scale=dinv[:, w:w + 1])
                nc.sync.dma_start(out=y1_sh[w * TILE:(w + 1) * TILE, :], in_=y_t[:])

            if stage >= 1:
                nc.gpsimd.collective_compute(
                    "AllGather", mybir.AluOpType.bypass, replica_groups=groups,
                    ins=[y1_sh.ap().opt()], outs=[y1_full.ap().opt()])

            # acc tiles for the epilogue reductions
            max_acc = accp.tile([TILE, 1], f32, tag="mx")
            nc.vector.memset(max_acc[:], -3e38)
            sum_acc = accp.tile([TILE, 1], f32, tag="sm")
            nc.vector.memset(sum_acc[:], 0.0)

            # ---------------- message-passing layers ----------------
            def layer(y_full, first):
                idx_col = 0
                tile0 = 0
                for w in range(W):
                    ntw_w = ntw[w]
                    dw = TILE if w < W - 1 else cfg.last_w
                    ps = mmp.tile([TILE, D], f32, tag="agg")
                    if ntw_w:
                        it = iop.tile([TILE, maxtw * 8], i16, tag="idx")
                        nc.sync.dma_start(
                            out=it[:, :ntw_w * 8],
                            in_=idx_w[:, idx_col:idx_col + ntw_w * 8])
                        dt = iop.tile([TILE, maxtw], bf16, tag="dst")
                        nc.sync.dma_start(
                            out=dt[:, :ntw_w],
                            in_=dest_t[:, tile0:tile0 + ntw_w])
                        mt = msgp.tile([TILE, maxtw * TILE], bf16, tag="msg")
                        st = sp.tile([TILE, maxtw * TILE], bf16, tag="S")
                        toff = 0
                        for qq in range(NQ):
                            tq = T[w][qq]
                            if tq == 0:
                                continue
                            nidx = tq * TILE
                            nc.gpsimd.dma_gather(
                                out_ap=mt[:, toff * TILE:(toff + tq) * TILE]
                                    .rearrange("p (t e) -> p t e", e=TILE),
                                in_ap=y_full[qq * QROWS:(qq + 1) * QROWS, :],
                                idxs_ap=it[:, toff * 8:(toff + tq) * 8],
                                num_idxs=nidx,
                                num_idxs_reg=nidx,
                                elem_size=D,
                                # single-packet mode caps at 64 descriptors
                                # per SDMA engine = 1024 indices
                                single_packet=(nidx <= 1024),
                            )
                            toff += tq
                        # S[p, t, j] = (iota[j] == dest[p, t])
                        nc.vector.tensor_tensor(
                            out=st[:, :ntw_w * TILE]
                                .rearrange("p (t e) -> p t e", e=TILE),
                            in0=iota_t[:].unsqueeze(1)
                                .to_broadcast([TILE, ntw_w, TILE]),
                            in1=dt[:, :ntw_w].unsqueeze(2)
                                .to_broadcast([TILE, ntw_w, TILE]),
                            op=OP.is_equal)
                        for t in range(ntw_w):
                            nc.tensor.matmul(
                                ps[:],
                                lhsT=st[:, t * TILE:(t + 1) * TILE],
                                rhs=mt[:, t * TILE:(t + 1) * TILE],
                                start=(t == 0), stop=(t == ntw_w - 1))
                        idx_col += ntw_w * 8
                        tile0 += ntw_w

                    slw = sl[:, w * D:(w + 1) * D]
                    h_t = flp.tile([TILE, D], f32, tag="h")
                    if ntw_w:
                        # h = agg * dinv + sl
                        nc.vector.tensor_scalar(out=h_t[:], in0=ps[:],
                                                scalar1=dinv[:, w:w + 1],
                                                scalar2=None, op0=OP.mult)
                        nc.vector.tensor_tensor(out=h_t[:], in0=h_t[:],
                                                in1=slw, op=OP.add)
                    else:
                        nc.vector.tensor_copy(out=h_t[:], in_=slw)
                    if first:
                        nc.scalar.activation(h_t[:], h_t[:], AF.Relu)
                        # hw2 = h1 @ W2 (transpose + matmul, like prologue)
                        hT_ps = trp.tile([TILE, D], f32, tag="tr")
                        nc.tensor.transpose(hT_ps[:], h_t[:], ident[:])
                        hT = flp.tile([TILE, D], f32, tag="hT")
                        nc.vector.tensor_copy(out=hT[:], in_=hT_ps[:])
                        hw_ps = mmp.tile([TILE, D], f32, tag="hw")
                        nc.tensor.matmul(hw_ps[:], lhsT=hT[:], rhs=w2_t[:],
                                         start=True, stop=True)
                        # y2 = hw2 * dinv (bf16)
                        y_t = flp.tile([TILE, D], bf16, tag="y2")
                        nc.scalar.activation(y_t[:], hw_ps[:], AF.Copy,
                                             scale=dinv[:, w:w + 1])
                        nc.sync.dma_start(
                            out=y2_sh[w * TILE:(w + 1) * TILE, :], in_=y_t[:])
                        # sl2 = hw2 * dinv2 + b2 (in place over sl)
                        nc.vector.tensor_scalar(out=slw, in0=hw_ps[:],
                                                scalar1=dinv2[:, w:w + 1],
                                                scalar2=None, op0=OP.mult)
                        nc.vector.tensor_tensor(out=slw, in0=slw,
                                                in1=b2_bc[:], op=OP.add)
                    else:
                        nc.sync.dma_start(
                            out=h2_sh[w * TILE:(w + 1) * TILE, :], in_=h_t[:])
                        # track per-feature max over this window's rows
                        hT_ps = trp.tile([TILE, D], f32, tag="tr")
                        nc.tensor.transpose(hT_ps[:], h_t[:], ident[:])
                        mx_t = flp.tile([TILE, 1], f32, tag="mx_t")
                        nc.vector.tensor_reduce(
                            out=mx_t[:], in_=hT_ps[:, :dw],
                            axis=mybir.AxisListType.X, op=OP.max)
                        nc.vector.tensor_tensor(out=max_acc[:], in0=max_acc[:],
                                                in1=mx_t[:], op=OP.max)

            if stage >= 2:
                layer(y1_full, True)
            if stage >= 3:
                nc.gpsimd.collective_compute(
                    "AllGather", mybir.AluOpType.bypass, replica_groups=groups,
                    ins=[y2_sh.ap().opt()], outs=[y2_full.ap().opt()])
                layer(y2_full, False)

            if stage < 4:
                for w in range(W):
                    o_t = flp.tile([TILE, D], f32, tag="o")
                    nc.vector.tensor_copy(out=o_t[:], in_=sl[:, w * D:(w + 1) * D])
                    nc.sync.dma_start(out=out_sh[w * TILE:(w + 1) * TILE, :],
                                      in_=o_t[:])
            else:
                # ---------------- log-softmax epilogue ----------------

                # global max across cores: max_acc [128,1] -> row -> AllReduce
                mxT_ps = trp.tile([TILE, TILE], f32, tag="tr")
                nc.tensor.transpose(mxT_ps[:1, :TILE], max_acc[:], ident[:])
                mx_row = flp.tile([1, D], f32, tag="mxrow")
                nc.vector.tensor_copy(out=mx_row[:], in_=mxT_ps[:1, :TILE])
                nc.sync.dma_start(out=red_in[:, :], in_=mx_row[:])
                nc.gpsimd.collective_compute(
                    "AllReduce", mybir.AluOpType.max, replica_groups=groups,
                    ins=[red_in.ap().opt()], outs=[red_max.ap().opt()])
                gmax_row = constp.tile([1, D], f32, tag="gmaxrow")
                nc.sync.dma_start(out=gmax_row[:], in_=red_max[:, :])
                # per-partition column ngmax[p] = -gmax[p] via K=1 matmul
                # (out[m, 0] = lhsT[0, m] * ones[0, 0])
                gcol_ps = trp.tile([TILE, TILE], f32, tag="tr")
                nc.tensor.matmul(gcol_ps[:TILE, :1], lhsT=gmax_row[:],
                                 rhs=ones_row[:, :1], start=True, stop=True)
                ngmax = constp.tile([TILE, 1], f32, tag="ngmax")
                nc.vector.tensor_scalar(out=ngmax[:], in0=gcol_ps[:TILE, :1],
                                        scalar1=-1.0, scalar2=None, op0=OP.mult)

                # sum of exp(h2 - gmax) per feature
                for w in range(W):
                    dw = TILE if w < W - 1 else cfg.last_w
                    h_t = flp.tile([TILE, D], f32, tag="h2r")
                    nc.sync.dma_start(out=h_t[:],
                                      in_=h2_sh[w * TILE:(w + 1) * TILE, :])
                    hT_ps = trp.tile([TILE, D], f32, tag="tr")
                    nc.tensor.transpose(hT_ps[:], h_t[:], ident[:])
                    e_t = flp.tile([TILE, D], f32, tag="e")
                    se_t = flp.tile([TILE, 1], f32, tag="se")
                    nc.scalar.activation(e_t[:, :dw], hT_ps[:, :dw], AF.Exp,
                                         bias=ngmax[:], accum_out=se_t[:])
                    nc.vector.tensor_tensor(out=sum_acc[:], in0=sum_acc[:],
                                            in1=se_t[:], op=OP.add)

                smT_ps = trp.tile([TILE, TILE], f32, tag="tr")
                nc.tensor.transpose(smT_ps[:1, :TILE], sum_acc[:], ident[:])
                sm_row = flp.tile([1, D], f32, tag="smrow")
                nc.vector.tensor_copy(out=sm_row[:], in_=smT_ps[:1, :TILE])
                nc.sync.dma_start(out=red_in2[:, :], in_=sm_row[:])
                nc.gpsimd.collective_compute(
                    "AllReduce", mybir.AluOpType.add, replica_groups=groups,
                    ins=[red_in2.ap().opt()], outs=[red_sum.ap().opt()])
                gsum_row = constp.tile([1, D], f32, tag="gsumrow")
                nc.sync.dma_start(out=gsum_row[:], in_=red_sum[:, :])
                lg_row = constp.tile([1, D], f32, tag="lgrow")
                nc.scalar.activation(lg_row[:], gsum_row[:], AF.Ln)
                corr_row = constp.tile([1, D], f32, tag="corrrow")
                nc.vector.tensor_tensor(out=corr_row[:], in0=lg_row[:],
                                        in1=gmax_row[:], op=OP.add)
                cb_ps = trp.tile([TILE, D], f32, tag="tr")
                nc.tensor.matmul(cb_ps[:], lhsT=ones_row[:], rhs=corr_row[:],
                                 start=True, stop=True)
                corr_bc = constp.tile([TILE, D], f32, tag="corrbc")
                nc.vector.tensor_copy(out=corr_bc[:], in_=cb_ps[:])

                for w in range(W):
                    h_t = flp.tile([TILE, D], f32, tag="h2f")
                    nc.sync.dma_start(out=h_t[:],
                                      in_=h2_sh[w * TILE:(w + 1) * TILE, :])
                    o_t = flp.tile([TILE, D], f32, tag="o")
                    nc.vector.tensor_tensor(out=o_t[:], in0=h_t[:],
                                            in1=corr_bc[:], op=OP.subtract)
                    nc.sync.dma_start(out=out_sh[w * TILE:(w + 1) * TILE, :],
                                      in_=o_t[:])

    nc.finalize()
    return nc


# --------------------------------------------------------------------------
# runner
# --------------------------------------------------------------------------

_CACHE = {}


def get_program(cfg, meta, stage=4):
    key = (cfg.N, cfg.E, cfg.D, cfg.NC, meta["T"], stage)
    if key not in _CACHE:
        _CACHE[key] = build(cfg, meta, stage=stage)
    return _CACHE[key]


def run(cfg, in_maps, meta):
    from concourse.bass_utils import run_bass_kernel_spmd
    nc = get_program(cfg, meta)
    res = run_bass_kernel_spmd(nc, in_maps, list(range(cfg.NC)))
    return [r["out_sh"] for r in res.results]


def _kernel_impl(cfg, x, edge_index, question_embeddings, W1, b1, W2, b2,
                 Wq, bq):
    in_maps, meta = prepare(cfg, x, edge_index, W1, b1, W2, b2)
    outs = run(cfg, in_maps, meta)
    h = np.concatenate([o[:cfg.NPC] for o in outs], axis=0)
    B = np.asarray(question_embeddings).shape[0]
    out = np.broadcast_to(h[None, :, :], (B, cfg.N, cfg.D))
    return np.ascontiguousarray(out, dtype=np.float32)


def kernel(x, edge_index, question_embeddings, W1, b1, W2, b2, Wq, bq):
    cfg = Cfg(x.shape[0], edge_index.shape[1], x.shape[1])
    return _kernel_impl(cfg, np.asarray(x), np.asarray(edge_index),
                        np.asarray(question_embeddings),
                        np.asarray(W1), np.asarray(b1),
                        np.asarray(W2), np.asarray(b2),
                        np.asarray(Wq), np.asarray(bq))

